# revision 32
# baseline (speedup 1.0000x reference)
"""AngularAggLayer Trainium2 kernel — 8-core row-sharded, fp8 DoubleRow,
blended Karatsuba complex multiply, 3-queue DMA streaming, host epilogue.

Host (numpy) does the cheap prep: normalized features, class centers, fake
labels, and the masked angle-modulation planes
  AC = A_bin*(cos(W)-1), AS = A_bin*sin(W), ACS = AC + AS
quantized to fp8e4m3 per core slab. Each NeuronCore accumulates the three
Karatsuba partial products of the complex message correction
  B1 = nfr.T @ AC, B2 = nfi.T @ AS, B3 = (nfr+nfi).T @ ACS
with fp8 DoubleRow matmuls (256-row contraction tiles) into six PSUM banks
(2 column chunks x 3 planes). For the first N2 contraction tiles the ACS
plane is not shipped; B3 instead takes two matmuls against AC and AS with
the summed weights — 2 adjacency planes / 8 matmuls vs 3 planes / 6
matmuls — which balances the PE stream (11.5-13.6us) against the three
concurrent DMA-issuing queues (sync / scalar / gpsimd, ~15.1us each).

The tail is one PSUM->SBUF bf16 copy per output plane (ACT and DVE are the
only engines with a PSUM port; ACT's activation table is pre-warmed during
the load phase) and six bf16 out DMAs spread over the queues. The exit
barrier's wait-carrying Drains are softened to EventSemaphores so the
kernel completes on data-landed DMA lane semaphores + compute-done engine
semaphores instead of the DGE pipeline's post-transfer drain latency.

The host reassembles corr_r = B1-B2, corr_i = B3-B1-B2, adds the exact
host-computed column-sum (the "+1" of e^{i*0}=1 on non-edges), and
normalizes to unit modulus. All O(N^2 D) work stays on the device; host
prep/epilogue are O(N D + E + N C) elementwise.
"""

import numpy as np

N, D, C = 6144, 128, 16
NCORES = 8
NS = N // NCORES          # 768 rows per core
K2 = N // 256             # 24 DoubleRow contraction tiles of 256
NCH = 2                   # output column chunks
MC = NS // NCH            # 384 columns per chunk
EPS = np.float32(1e-5)

N2 = 11                   # first N2 tiles ship 2 adjacency planes (no ACS)

DMA_RATE = 0.3855         # ns per byte-per-partition (DMA_CYCLE)
DMA_MIN = 500.0           # descriptor-gen floor per transfer

_CACHE = {}


def _legalize_waits(nc, mybir, max_waits=1):
    """Walrus in this container accepts only one sem wait per instruction;
    spill extras onto NoOps inserted just before, on the same engine."""
    ctr = 0
    for f in nc.m.functions:
        for bb in f.blocks:
            out, changed = [], False
            for inst in bb.instructions:
                si = inst.sync_info
                waits = list(si.on_wait) if si is not None and si.on_wait else []
                if len(waits) > max_waits:
                    while len(waits) > max_waits:
                        chunk, waits = waits[:max_waits], waits[max_waits:]
                        nop = mybir.InstNoOp(name=f"waitnop-{ctr}", ins=[], outs=[])
                        ctr += 1
                        nop.engine = inst.engine
                        nop.sync_info = mybir.SyncInfo(on_wait=chunk, on_update=[])
                        out.append(nop)
                    si.on_wait = waits
                    changed = True
                out.append(inst)
            if changed:
                bb.instructions = out


def _strip_final_barrier(nc, mybir):
    """TileContext exit emits two sequential all-engine barrier rounds;
    the first already drains every engine and fans in all DMA-completion
    semaphores, so the trailing round is redundant — drop it."""
    for f in nc.m.functions:
        for bb in f.blocks:
            ins = bb.instructions
            cut = len(ins)
            while cut > 0 and type(ins[cut - 1]).__name__ in (
                    "InstDrain", "InstEventSemaphore"):
                cut -= 1
            if cut < len(ins):
                bb.instructions = ins[:cut]


def _reorder_fanin_waits(nc):
    """The exit drain's legalized wait-NoOps dispatch in list order; put the
    late-resolving semaphores (the DMAHW lanes that the final output DMAs
    land on) last so the earlier NoOps dispatch while the last DMA completes
    instead of after it."""
    for f in nc.m.functions:
        for bb in f.blocks:
            for inst in bb.instructions:
                si = inst.sync_info
                if si is None or not si.on_wait or len(si.on_wait) < 8:
                    continue
                def key(w):
                    nm = getattr(w, "ant_name", "") or ""
                    v = getattr(w, "wait_value", 0) or 0
                    if not nm.startswith("DMAHW"):
                        return (0, nm)
                    return (1, v, nm)
                si.on_wait = sorted(si.on_wait, key=key)


def _soften_exit_drains(nc, mybir):
    """The exit barrier's per-engine Drains stall until every prior
    instruction fully completes — for the final output DMAs that includes
    the DGE pipeline's post-transfer drain latency, long after the data has
    landed. The barrier's wait list already fans in every DMA lane
    semaphore (the data-landed signal SDMA bumps after the last descriptor
    writes) plus the per-engine compute semaphores, so replace the
    wait-carrying Drains with EventSemaphores keeping identical sync_info:
    the barrier then releases on data-landed + compute-done. Bare Drains
    (no waits) are kept."""
    ctr = 0
    for f in nc.m.functions:
        bb = f.blocks[-1]
        out = []
        for i in bb.instructions:
            si = i.sync_info
            if type(i).__name__ == "InstDrain" and si is not None and si.on_wait:
                sem = mybir.InstEventSemaphore(name=f"exitsem-{ctr}",
                                               ins=[], outs=[])
                ctr += 1
                sem.engine = i.engine
                sem.sync_info = si
                out.append(sem)
            else:
                out.append(i)
        bb.instructions = out


def _build(legalize=True, cfg=None):
    import concourse.bass as bass
    import concourse.mybir as mybir
    from concourse import tile

    cfg = cfg or {}
    n2 = cfg.get("n2", N2)

    F32 = mybir.dt.float32
    F8 = mybir.dt.float8e4

    DR = mybir.MatmulPerfMode.DoubleRow

    nc = bass.Bass()
    # adjacency planes in device layout: [128, K2, 3(pl), 2(i), NS]
    acs_d = nc.declare_dram_parameter("acs", [128, K2 * 3 * 2 * NS], F8,
                                      isOutput=False)
    acs_r = acs_d.rearrange("p (t pl i n) -> p t pl i n", t=K2, pl=3, i=2)
    # nf planes in device layout: [128, K2, 3(pl), 2(i), D]
    nf_d = nc.declare_dram_parameter("nf", [128, K2 * 3 * 2 * D], F8,
                                     isOutput=False)
    nf_r = nf_d.rearrange("p (t pl i d) -> p t pl i d", t=K2, pl=3, i=2)
    BF16 = mybir.dt.bfloat16
    # raw bf16 partial-product planes out: [D, NCH, 3, MC]
    out_d = nc.declare_dram_parameter("out", [D, NCH * 3 * MC], BF16,
                                      isOutput=True)
    out_r = out_d.rearrange("d (c pl n) -> d c pl n", c=NCH, pl=3)

    with tile.TileContext(nc) as tc:
        with (
            tc.tile_pool(name="const", bufs=1) as const,
            tc.tile_pool(name="psM", bufs=1, space="PSUM") as psM,
        ):
            # ---- resident operands ----
            nf_w = const.tile([128, K2, 3, 2, D], F8)
            adj = const.tile([128, K2, 3, 2, NS], F8)

            # ---- DMA job list in PE-consumption priority order ----
            nfb = 3 * 2 * D          # bytes/partition per nf k2-tile
            adb = 2 * NS             # bytes/partition per adjacency plane
            jobs = []
            nf_groups = [(0, 1), (1, 4), (4, 12), (12, 24)]
            nfi = 0

            def push_nf(t):
                nonlocal nfi
                while nfi < len(nf_groups) and nf_groups[nfi][0] <= t:
                    lo, hi = nf_groups[nfi]
                    jobs.append(("nf", lo, hi))
                    nfi += 1

            for t in range(K2):
                push_nf(t)
                nplanes = 2 if t < n2 else 3
                for pl in range(nplanes):
                    jobs.append(("adj", t, pl))

            # Greedy earliest-finish assignment over the three DMA-issuing
            # queues. Each queue serializes its transfers; queues run
            # concurrently. Cost model: max(500, bytes_per_partition*0.3855).
            engs = [nc.sync, nc.scalar, nc.gpsimd]
            # bias the scalar (ACT) queue by the act-table warm-up it must
            # run after its input share, so all queues drain together
            load = [200.0, 200.0 + 1383.0, 100.0]
            for job in jobs:
                if job[0] == "nf":
                    _, lo, hi = job
                    dst, src = nf_w[:, lo:hi], nf_r[:, lo:hi]
                    b = (hi - lo) * nfb
                else:
                    _, t, pl = job
                    dst = adj[:, t, pl]
                    src = acs_r[:, t, pl]
                    b = adb
                qi = load.index(min(load))
                engs[qi].dma_start(dst, src)
                load[qi] += max(DMA_MIN, b * DMA_RATE)

            # preload the ACT function table (Copy) before the tail needs
            # it — the implicit load costs ~1.4us. Emitted after the DMA
            # issue loop so it queues behind ACT's input transfers.
            warm = const.tile([128, 1], F32)
            nc.vector.memset(warm[:], 0.0)
            nc.scalar.copy(warm[:], warm[:])

            # ---- persistent accumulators: 6 PSUM banks ----
            ps = [[psM.tile([128, 512], F32, tag=f"ps{c}{p}", name=f"ps{c}{p}")
                   for p in range(3)] for c in range(NCH)]

            def mm(c, pl, t, wpl, apl, start, stop):
                cs = slice(c * MC, (c + 1) * MC)
                nc.tensor.matmul(ps[c][pl][:, 0:MC], nf_w[:, t, wpl],
                                 adj[:, t, apl][:, :, cs],
                                 start=start, stop=stop, perf_mode=DR)

            # plane roles: B1 = nfr@AC, B2 = nfi@AS, B3 = (nfr+nfi)@ACS
            # (or, for 2-plane tiles, B3 = (nfr+nfi)@AC + (nfr+nfi)@AS)
            grp = cfg.get("grp", 2)   # final tiles emitted chunk-major
            g0 = K2 - grp

            def emit_tile_chunk(t, c, stop):
                two = t < n2
                first = (t == 0)
                mm(c, 0, t, 0, 0, first, stop)
                mm(c, 1, t, 1, 1, first, stop)
                if two:
                    mm(c, 2, t, 2, 0, first, False)
                    mm(c, 2, t, 2, 1, False, stop)
                else:
                    mm(c, 2, t, 2, 2, first, stop)

            for t in range(g0):
                first, two = (t == 0), (t < n2)
                for c in range(NCH):
                    mm(c, 0, t, 0, 0, first, False)
                for c in range(NCH):
                    mm(c, 1, t, 1, 1, first, False)
                if two:
                    for c in range(NCH):
                        mm(c, 2, t, 2, 0, first, False)
                    for c in range(NCH):
                        mm(c, 2, t, 2, 1, False, False)
                else:
                    for c in range(NCH):
                        mm(c, 2, t, 2, 2, first, False)
            # final `grp` tiles chunk-major: all of chunk 0 first so its
            # banks close early and copies overlap chunk 1's matmuls
            for c in range(NCH):
                for t in range(g0, K2):
                    emit_tile_chunk(t, c, stop=(t == K2 - 1))

            # ---- tail: PSUM -> SBUF copies (only ACT/DVE can read PSUM),
            # then out DMAs ----
            ot = [[const.tile([128, MC], BF16, tag=f"ot{c}{p}",
                              name=f"ot{c}{p}") for p in range(3)]
                  for c in range(NCH)]
            # chunk-0 copies run while chunk 1 still matmuls; chunk 1's
            # planes close 80ns apart right at the stream end, so they are
            # interleaved across ACT/DVE in close order. Early out DMAs ride
            # Pool (slower SWDGE completion), late ones SP/ACT (HWDGE).
            nc.scalar.copy(ot[0][0][:], ps[0][0][:, 0:MC])
            nc.vector.tensor_scalar_add(ot[0][1][:], ps[0][1][:, 0:MC], 0.0)
            nc.scalar.copy(ot[0][2][:], ps[0][2][:, 0:MC])
            nc.vector.tensor_scalar_add(ot[1][0][:], ps[1][0][:, 0:MC], 0.0)
            nc.scalar.copy(ot[1][1][:], ps[1][1][:, 0:MC])
            nc.vector.tensor_scalar_add(ot[1][2][:], ps[1][2][:, 0:MC], 0.0)

            nc.sync.dma_start(out_r[:, 0, 0], ot[0][0][:])
            nc.gpsimd.dma_start(out_r[:, 0, 1], ot[0][1][:])
            nc.sync.dma_start(out_r[:, 0, 2], ot[0][2][:])
            nc.gpsimd.dma_start(out_r[:, 1, 0], ot[1][0][:])
            nc.scalar.dma_start(out_r[:, 1, 1], ot[1][1][:])
            nc.sync.dma_start(out_r[:, 1, 2], ot[1][2][:])

    _strip_final_barrier(nc, mybir)
    _soften_exit_drains(nc, mybir)
    _reorder_fanin_waits(nc)
    if legalize:
        _legalize_waits(nc, mybir)
    return nc


def _get_nc():
    if "nc" not in _CACHE:
        _CACHE["nc"] = _build()
    return _CACHE["nc"]


def _host_prep(x_real, x_imag, A, theta, params_real, params_imag, labels):
    import ml_dtypes

    FP8 = ml_dtypes.float8_e4m3fn
    x_real = np.asarray(x_real, np.float32)
    x_imag = np.asarray(x_imag, np.float32)
    A = np.asarray(A, np.float32)
    theta = np.asarray(theta, np.float32)
    labels = np.asarray(labels)

    # --- host prep (mirrors reference order in float32) ---
    x = (x_real + 1j * x_imag).astype(np.complex64)
    nf = x / (np.abs(x) + EPS)                      # [N, D] complex64
    one_hot = np.zeros((N, C), np.float32)
    one_hot[np.arange(N), labels] = 1.0
    sum_by_label = np.einsum("nc,nd->cd", one_hot.astype(np.complex64), nf)
    counts = one_hot.sum(axis=0)[:, None]
    mean_tensor = sum_by_label / counts             # [C, D] complex64

    params = (np.asarray(params_real, np.float32)
              + 1j * np.asarray(params_imag, np.float32)).astype(np.complex64)
    p1, p2 = params[:D], params[D:]
    s_feat = nf @ p1                                # [N, 1]
    s_cent = mean_tensor @ p2                       # [C, 1]
    scores = np.abs(s_feat[:, None, :] + s_cent[None, :, :])[..., 0]
    fl = np.argmax(scores, axis=1)                  # [N] fake labels

    iu = np.triu_indices(C, k=1)
    il = np.tril_indices(C, k=-1)
    M = np.zeros((C, C), np.float32)
    M[iu[0], iu[1]] = theta
    M[il[1], il[0]] = -theta
    Mcos = np.cos(M) - np.float32(1.0)   # cos(W)-1 (the +1 is folded into
    Msin = np.sin(M)                     # the host-side colsum correction)
    Mcs = Mcos + Msin                    # Karatsuba third plane

    # nf planes packed [128, K2, 3(pl), 2(i), D]; row m = t*256 + i*128 + p
    nfr, nfim = nf.real, nf.imag
    nf3 = np.stack([nfr, nfim, nfr + nfim], axis=1)        # [N, 3, D] f32
    nf_pack = np.ascontiguousarray(
        nf3.reshape(K2, 2, 128, 3, D).transpose(2, 0, 3, 1, 4).reshape(128, -1)
    ).astype(FP8)

    colsum = (nfr.sum(axis=0, dtype=np.float64)
              + 1j * nfim.sum(axis=0, dtype=np.float64))   # [D] complex128

    # --- masked fp8 modulation planes, per-core packed slab ---
    nn_, mm = np.nonzero(A)              # A[n, m] edges, sorted by n
    fln, flm = fl[nn_], fl[mm]
    vals_c = Mcos[fln, flm].astype(FP8)
    vals_s = Msin[fln, flm].astype(FP8)
    vals_cs = Mcs[fln, flm].astype(FP8)

    in_maps = []
    for cid in range(NCORES):
        lo, hi = np.searchsorted(nn_, [cid * NS, (cid + 1) * NS])
        n_loc = nn_[lo:hi] - cid * NS
        m_sel = mm[lo:hi]
        acs = np.zeros((N, 3, NS), FP8)
        acs[m_sel, 0, n_loc] = vals_c[lo:hi]
        acs[m_sel, 1, n_loc] = vals_s[lo:hi]
        acs[m_sel, 2, n_loc] = vals_cs[lo:hi]
        # pack to [128, K2, 3(pl), 2(i), NS]; row m = t*256 + i*128 + p
        acs_pack = np.ascontiguousarray(
            acs.reshape(K2, 2, 128, 3, NS).transpose(2, 0, 3, 1, 4)
            .reshape(128, -1))
        in_maps.append(dict(acs=acs_pack, nf=nf_pack))
    _CACHE["colsum"] = colsum
    return in_maps


def kernel(x_real, x_imag, A, theta, params_real, params_imag, labels):
    from concourse.bass_utils import run_bass_kernel_spmd

    in_maps = _host_prep(x_real, x_imag, A, theta, params_real, params_imag,
                         labels)
    nc = _get_nc()
    _CACHE["last_maps"] = in_maps
    res = run_bass_kernel_spmd(nc, in_maps, list(range(NCORES))).results

    colsum = _CACHE["colsum"]            # [D] complex128
    out = np.empty((N, D), np.complex64)
    for cid in range(NCORES):
        o = np.asarray(res[cid]["out"], np.float32).reshape(D, NCH, 3, MC)
        B = o.transpose(2, 0, 1, 3).reshape(3, D, NS)      # [3, D, NS]
        corr_r = (B[0] - B[1]).T.astype(np.float64)        # [NS, D]
        corr_i = (B[2] - B[0] - B[1]).T.astype(np.float64)
        msg = (corr_r + 1j * corr_i) + colsum[None, :]
        rows = slice(cid * NS, (cid + 1) * NS)
        out[rows] = (msg / (np.abs(msg) + float(EPS))).astype(np.complex64)
    return out


# revision 40
# speedup vs baseline: 1.0067x; 1.0067x over previous
"""AngularAggLayer Trainium2 kernel — 8-core row-sharded, fp8 DoubleRow,
blended Karatsuba complex multiply, 3-queue DMA streaming, host epilogue.

Host (numpy) does the cheap prep: normalized features, class centers, fake
labels, and the masked angle-modulation planes
  AC = A_bin*(cos(W)-1), AS = A_bin*sin(W), ACS = AC + AS
quantized to fp8e4m3 per core slab. Each NeuronCore accumulates the three
Karatsuba partial products of the complex message correction
  B1 = nfr.T @ AC, B2 = nfi.T @ AS, B3 = (nfr+nfi).T @ ACS
with fp8 DoubleRow matmuls (256-row contraction tiles) into six PSUM banks
(2 column chunks x 3 planes). For the first N2 contraction tiles the ACS
plane is not shipped; B3 instead takes two matmuls against AC and AS with
the summed weights — 2 adjacency planes / 8 matmuls vs 3 planes / 6
matmuls — which balances the PE stream (11.5-13.6us) against the three
concurrent DMA-issuing queues (sync / scalar / gpsimd, ~15.1us each).

The tail is one PSUM->SBUF bf16 copy per output plane (ACT and DVE are the
only engines with a PSUM port; ACT's activation table is pre-warmed during
the load phase) and six bf16 out DMAs spread over the queues. The exit
barrier's wait-carrying Drains are softened to EventSemaphores so the
kernel completes on data-landed DMA lane semaphores + compute-done engine
semaphores instead of the DGE pipeline's post-transfer drain latency.

The host reassembles corr_r = B1-B2, corr_i = B3-B1-B2, adds the exact
host-computed column-sum (the "+1" of e^{i*0}=1 on non-edges), and
normalizes to unit modulus. All O(N^2 D) work stays on the device; host
prep/epilogue are O(N D + E + N C) elementwise.
"""

import numpy as np

N, D, C = 6144, 128, 16
NCORES = 8
NS = N // NCORES          # 768 rows per core
K2 = N // 256             # 24 DoubleRow contraction tiles of 256
NCH = 2                   # output column chunks
MC = NS // NCH            # 384 columns per chunk
EPS = np.float32(1e-5)

N2 = 11                   # first N2 tiles ship 2 adjacency planes (no ACS)

DMA_RATE = 0.3855         # ns per byte-per-partition (DMA_CYCLE)
DMA_MIN = 500.0           # descriptor-gen floor per transfer

_CACHE = {}


def _legalize_waits(nc, mybir, max_waits=1):
    """Walrus in this container accepts only one sem wait per instruction;
    spill extras onto NoOps inserted just before, on the same engine."""
    ctr = 0
    for f in nc.m.functions:
        for bb in f.blocks:
            out, changed = [], False
            for inst in bb.instructions:
                si = inst.sync_info
                waits = list(si.on_wait) if si is not None and si.on_wait else []
                if len(waits) > max_waits:
                    while len(waits) > max_waits:
                        chunk, waits = waits[:max_waits], waits[max_waits:]
                        nop = mybir.InstNoOp(name=f"waitnop-{ctr}", ins=[], outs=[])
                        ctr += 1
                        nop.engine = inst.engine
                        nop.sync_info = mybir.SyncInfo(on_wait=chunk, on_update=[])
                        out.append(nop)
                    si.on_wait = waits
                    changed = True
                out.append(inst)
            if changed:
                bb.instructions = out


def _strip_final_barrier(nc, mybir):
    """TileContext exit emits two sequential all-engine barrier rounds;
    the first already drains every engine and fans in all DMA-completion
    semaphores, so the trailing round is redundant — drop it."""
    for f in nc.m.functions:
        for bb in f.blocks:
            ins = bb.instructions
            cut = len(ins)
            while cut > 0 and type(ins[cut - 1]).__name__ in (
                    "InstDrain", "InstEventSemaphore"):
                cut -= 1
            if cut < len(ins):
                bb.instructions = ins[:cut]


def _reorder_fanin_waits(nc):
    """The exit drain's legalized wait-NoOps dispatch in list order; put the
    late-resolving semaphores (the DMAHW lanes that the final output DMAs
    land on) last so the earlier NoOps dispatch while the last DMA completes
    instead of after it."""
    for f in nc.m.functions:
        for bb in f.blocks:
            for inst in bb.instructions:
                si = inst.sync_info
                if si is None or not si.on_wait or len(si.on_wait) < 8:
                    continue
                def key(w):
                    nm = getattr(w, "ant_name", "") or ""
                    v = getattr(w, "wait_value", 0) or 0
                    if not nm.startswith("DMAHW"):
                        return (0, nm)
                    return (1, v, nm)
                si.on_wait = sorted(si.on_wait, key=key)


def _soften_exit_drains(nc, mybir):
    """The exit barrier's per-engine Drains stall until every prior
    instruction fully completes — for the final output DMAs that includes
    the DGE pipeline's post-transfer drain latency, long after the data has
    landed. The barrier's wait list already fans in every DMA lane
    semaphore (the data-landed signal SDMA bumps after the last descriptor
    writes) plus the per-engine compute semaphores, so replace the
    wait-carrying Drains with EventSemaphores keeping identical sync_info:
    the barrier then releases on data-landed + compute-done. Bare Drains
    (no waits) are kept."""
    ctr = 0
    for f in nc.m.functions:
        bb = f.blocks[-1]
        out = []
        for i in bb.instructions:
            si = i.sync_info
            if type(i).__name__ == "InstDrain" and si is not None and si.on_wait:
                sem = mybir.InstEventSemaphore(name=f"exitsem-{ctr}",
                                               ins=[], outs=[])
                ctr += 1
                sem.engine = i.engine
                sem.sync_info = si
                out.append(sem)
            else:
                out.append(i)
        bb.instructions = out


def _build(legalize=True, cfg=None):
    import concourse.bass as bass
    import concourse.mybir as mybir
    from concourse import tile

    cfg = cfg or {}
    n2 = cfg.get("n2", N2)
    tail2 = cfg.get("tail2", False)

    def is2(t):
        return (t >= K2 - n2) if tail2 else (t < n2)

    F32 = mybir.dt.float32
    F8 = mybir.dt.float8e4

    DR = mybir.MatmulPerfMode.DoubleRow

    nc = bass.Bass()
    # adjacency planes in device layout: [128, K2, 3(pl), 2(i), NS]
    acs_d = nc.declare_dram_parameter("acs", [128, K2 * 3 * 2 * NS], F8,
                                      isOutput=False)
    acs_r = acs_d.rearrange("p (t pl i n) -> p t pl i n", t=K2, pl=3, i=2)
    # nf planes in device layout: [128, K2, 3(pl), 2(i), D]
    nf_d = nc.declare_dram_parameter("nf", [128, K2 * 3 * 2 * D], F8,
                                     isOutput=False)
    nf_r = nf_d.rearrange("p (t pl i d) -> p t pl i d", t=K2, pl=3, i=2)
    BF16 = mybir.dt.bfloat16
    # raw bf16 partial-product planes out: [D, NCH, 3, MC]
    out_d = nc.declare_dram_parameter("out", [D, NCH * 3 * MC], BF16,
                                      isOutput=True)
    out_r = out_d.rearrange("d (c pl n) -> d c pl n", c=NCH, pl=3)

    with tile.TileContext(nc) as tc:
        with (
            tc.tile_pool(name="const", bufs=1) as const,
            tc.tile_pool(name="psM", bufs=1, space="PSUM") as psM,
        ):
            # ---- resident operands ----
            nf_w = const.tile([128, K2, 3, 2, D], F8)
            adj = const.tile([128, K2, 3, 2, NS], F8)

            # ---- DMA job list in PE-consumption priority order ----
            nfb = 3 * 2 * D          # bytes/partition per nf k2-tile
            adb = 2 * NS             # bytes/partition per adjacency plane
            jobs = []
            nf_groups = [(0, 1), (1, 4), (4, 12), (12, 24)]
            nfi = 0

            def push_nf(t):
                nonlocal nfi
                while nfi < len(nf_groups) and nf_groups[nfi][0] <= t:
                    lo, hi = nf_groups[nfi]
                    jobs.append(("nf", lo, hi))
                    nfi += 1

            grp = cfg.get("grp", 2)   # final tiles emitted chunk-major
            g0 = K2 - grp
            ppm = ((cfg.get("ppm", False) or cfg.get("c0pp", False))
                   and g0 >= n2)
            for t in range(g0 if ppm else K2):
                push_nf(t)
                nplanes = 2 if is2(t) else 3
                for pl in range(nplanes):
                    jobs.append(("adj", t, pl))
            if ppm:
                # match the plane-pair-major consumption order of the
                # final tiles: ship plane p of every grp tile together
                push_nf(K2 - 1)
                for pl in range(3):
                    for t in range(g0, K2):
                        jobs.append(("adj", t, pl))

            # Greedy earliest-finish assignment over the three DMA-issuing
            # queues. Each queue serializes its transfers; queues run
            # concurrently. Cost model: max(500, bytes_per_partition*0.3855).
            engs = [nc.sync, nc.scalar, nc.gpsimd]
            # bias the scalar (ACT) queue by the act-table warm-up it must
            # run after its input share, so all queues drain together
            load = [200.0, 200.0 + 1383.0, 100.0]
            for job in jobs:
                if job[0] == "nf":
                    _, lo, hi = job
                    dst, src = nf_w[:, lo:hi], nf_r[:, lo:hi]
                    b = (hi - lo) * nfb
                else:
                    _, t, pl = job
                    dst = adj[:, t, pl]
                    src = acs_r[:, t, pl]
                    b = adb
                qi = load.index(min(load))
                engs[qi].dma_start(dst, src)
                load[qi] += max(DMA_MIN, b * DMA_RATE)

            # preload the ACT function table (Copy) before the tail needs
            # it — the implicit load costs ~1.4us. Emitted after the DMA
            # issue loop so it queues behind ACT's input transfers.
            warm = const.tile([128, 1], F32)
            nc.vector.memset(warm[:], 0.0)
            nc.scalar.copy(warm[:], warm[:])

            # ---- persistent accumulators: 6 PSUM banks ----
            ps = [[psM.tile([128, 512], F32, tag=f"ps{c}{p}", name=f"ps{c}{p}")
                   for p in range(3)] for c in range(NCH)]

            def mm(c, pl, t, wpl, apl, start, stop):
                cs = slice(c * MC, (c + 1) * MC)
                nc.tensor.matmul(ps[c][pl][:, 0:MC], nf_w[:, t, wpl],
                                 adj[:, t, apl][:, :, cs],
                                 start=start, stop=stop, perf_mode=DR)

            # plane roles: B1 = nfr@AC, B2 = nfi@AS, B3 = (nfr+nfi)@ACS
            # (or, for 2-plane tiles, B3 = (nfr+nfi)@AC + (nfr+nfi)@AS)

            def emit_tile_chunk(t, c, stop):
                two = is2(t)
                first = (t == 0)
                mm(c, 0, t, 0, 0, first, stop)
                mm(c, 1, t, 1, 1, first, stop)
                if two:
                    mm(c, 2, t, 2, 0, first, False)
                    mm(c, 2, t, 2, 1, False, stop)
                else:
                    mm(c, 2, t, 2, 2, first, stop)

            for t in range(g0):
                first, two = (t == 0), is2(t)
                for c in range(NCH):
                    mm(c, 0, t, 0, 0, first, False)
                for c in range(NCH):
                    mm(c, 1, t, 1, 1, first, False)
                if two:
                    for c in range(NCH):
                        mm(c, 2, t, 2, 0, first, False)
                    for c in range(NCH):
                        mm(c, 2, t, 2, 1, False, False)
                else:
                    for c in range(NCH):
                        mm(c, 2, t, 2, 2, first, False)
            if cfg.get("c0pp", False) and g0 >= n2:
                # pair chunk 0's planes across the final tiles so its three
                # regions close 10/8/6 matmuls before the end; chunk 1 stays
                # chunk-major (its stops are end-bound anyway)
                for pl in range(3):
                    for t in range(g0, K2):
                        mm(0, pl, t, pl, pl, False, t == K2 - 1)
                for t in range(g0, K2):
                    emit_tile_chunk(t, 1, stop=(t == K2 - 1))
            elif ppm:
                # plane-pair-major final region: interleave the last `grp`
                # tiles per (chunk, plane) so the six PSUM regions close
                # evenly spread (one per grp*80ns) instead of in two bursts
                # — the tail copies pipeline off each close immediately
                for c in range(NCH):
                    for pl in range(3):
                        for t in range(g0, K2):
                            mm(c, pl, t, pl, pl, False, t == K2 - 1)
            else:
                # chunk-major: all of chunk 0 first so its banks close
                # early and copies overlap chunk 1's matmuls
                for c in range(NCH):
                    for t in range(g0, K2):
                        emit_tile_chunk(t, c, stop=(t == K2 - 1))

            # ---- tail: PSUM -> SBUF copies (only ACT/DVE can read PSUM),
            # then out DMAs ----
            ot = [[const.tile([128, MC], BF16, tag=f"ot{c}{p}",
                              name=f"ot{c}{p}") for p in range(3)]
                  for c in range(NCH)]
            def cp(eng, c, p):
                if eng is nc.scalar:
                    nc.scalar.copy(ot[c][p][:], ps[c][p][:, 0:MC])
                else:
                    nc.vector.tensor_scalar_add(ot[c][p][:],
                                                ps[c][p][:, 0:MC], 0.0)

            if ppm:
                # regions close in order c0p0..c1p2; alternate DVE/ACT with
                # the slower DVE taking the first-closing plane. The last
                # plane is copied AND DMA'd by ACT so its out transfer
                # dispatches in-queue right after the copy retires.
                cp(nc.vector, 0, 0)
                cp(nc.scalar, 0, 1)
                cp(nc.vector, 0, 2)
                cp(nc.scalar, 1, 0)
                cp(nc.vector, 1, 1)
                cp(nc.scalar, 1, 2)
                nc.sync.dma_start(out_r[:, 0, 0], ot[0][0][:])
                nc.gpsimd.dma_start(out_r[:, 0, 1], ot[0][1][:])
                nc.sync.dma_start(out_r[:, 0, 2], ot[0][2][:])
                nc.gpsimd.dma_start(out_r[:, 1, 0], ot[1][0][:])
                nc.sync.dma_start(out_r[:, 1, 1], ot[1][1][:])
                nc.scalar.dma_start(out_r[:, 1, 2], ot[1][2][:])
            else:
                # chunk-0 copies run while chunk 1 still matmuls; chunk 1's
                # planes close 80ns apart at the stream end, interleaved
                # across ACT/DVE in close order. Early out DMAs ride Pool
                # (slower SWDGE completion), late ones SP/ACT (HWDGE).
                cp(nc.vector, 0, 0)
                cp(nc.scalar, 0, 1)
                cp(nc.vector, 0, 2)
                cp(nc.scalar, 1, 0)
                cp(nc.vector, 1, 1)
                cp(nc.scalar, 1, 2)
                nc.sync.dma_start(out_r[:, 0, 0], ot[0][0][:])
                nc.gpsimd.dma_start(out_r[:, 0, 1], ot[0][1][:])
                nc.sync.dma_start(out_r[:, 0, 2], ot[0][2][:])
                nc.gpsimd.dma_start(out_r[:, 1, 0], ot[1][0][:])
                nc.sync.dma_start(out_r[:, 1, 1], ot[1][1][:])
                nc.scalar.dma_start(out_r[:, 1, 2], ot[1][2][:])

    _strip_final_barrier(nc, mybir)
    _soften_exit_drains(nc, mybir)
    _reorder_fanin_waits(nc)
    if legalize:
        _legalize_waits(nc, mybir)
    return nc


def _get_nc():
    if "nc" not in _CACHE:
        _CACHE["nc"] = _build()
    return _CACHE["nc"]


def _host_prep(x_real, x_imag, A, theta, params_real, params_imag, labels):
    import ml_dtypes

    FP8 = ml_dtypes.float8_e4m3fn
    x_real = np.asarray(x_real, np.float32)
    x_imag = np.asarray(x_imag, np.float32)
    A = np.asarray(A, np.float32)
    theta = np.asarray(theta, np.float32)
    labels = np.asarray(labels)

    # --- host prep (mirrors reference order in float32) ---
    x = (x_real + 1j * x_imag).astype(np.complex64)
    nf = x / (np.abs(x) + EPS)                      # [N, D] complex64
    one_hot = np.zeros((N, C), np.float32)
    one_hot[np.arange(N), labels] = 1.0
    sum_by_label = np.einsum("nc,nd->cd", one_hot.astype(np.complex64), nf)
    counts = one_hot.sum(axis=0)[:, None]
    mean_tensor = sum_by_label / counts             # [C, D] complex64

    params = (np.asarray(params_real, np.float32)
              + 1j * np.asarray(params_imag, np.float32)).astype(np.complex64)
    p1, p2 = params[:D], params[D:]
    s_feat = nf @ p1                                # [N, 1]
    s_cent = mean_tensor @ p2                       # [C, 1]
    scores = np.abs(s_feat[:, None, :] + s_cent[None, :, :])[..., 0]
    fl = np.argmax(scores, axis=1)                  # [N] fake labels

    iu = np.triu_indices(C, k=1)
    il = np.tril_indices(C, k=-1)
    M = np.zeros((C, C), np.float32)
    M[iu[0], iu[1]] = theta
    M[il[1], il[0]] = -theta
    Mcos = np.cos(M) - np.float32(1.0)   # cos(W)-1 (the +1 is folded into
    Msin = np.sin(M)                     # the host-side colsum correction)
    Mcs = Mcos + Msin                    # Karatsuba third plane

    # nf planes packed [128, K2, 3(pl), 2(i), D]; row m = t*256 + i*128 + p
    nfr, nfim = nf.real, nf.imag
    nf3 = np.stack([nfr, nfim, nfr + nfim], axis=1)        # [N, 3, D] f32
    nf_pack = np.ascontiguousarray(
        nf3.reshape(K2, 2, 128, 3, D).transpose(2, 0, 3, 1, 4).reshape(128, -1)
    ).astype(FP8)

    colsum = (nfr.sum(axis=0, dtype=np.float64)
              + 1j * nfim.sum(axis=0, dtype=np.float64))   # [D] complex128

    # --- masked fp8 modulation planes, per-core packed slab ---
    nn_, mm = np.nonzero(A)              # A[n, m] edges, sorted by n
    fln, flm = fl[nn_], fl[mm]
    vals_c = Mcos[fln, flm].astype(FP8)
    vals_s = Msin[fln, flm].astype(FP8)
    vals_cs = Mcs[fln, flm].astype(FP8)

    in_maps = []
    for cid in range(NCORES):
        lo, hi = np.searchsorted(nn_, [cid * NS, (cid + 1) * NS])
        n_loc = nn_[lo:hi] - cid * NS
        m_sel = mm[lo:hi]
        acs = np.zeros((N, 3, NS), FP8)
        acs[m_sel, 0, n_loc] = vals_c[lo:hi]
        acs[m_sel, 1, n_loc] = vals_s[lo:hi]
        acs[m_sel, 2, n_loc] = vals_cs[lo:hi]
        # pack to [128, K2, 3(pl), 2(i), NS]; row m = t*256 + i*128 + p
        acs_pack = np.ascontiguousarray(
            acs.reshape(K2, 2, 128, 3, NS).transpose(2, 0, 3, 1, 4)
            .reshape(128, -1))
        in_maps.append(dict(acs=acs_pack, nf=nf_pack))
    _CACHE["colsum"] = colsum
    return in_maps


def kernel(x_real, x_imag, A, theta, params_real, params_imag, labels):
    from concourse.bass_utils import run_bass_kernel_spmd

    in_maps = _host_prep(x_real, x_imag, A, theta, params_real, params_imag,
                         labels)
    nc = _get_nc()
    _CACHE["last_maps"] = in_maps
    res = run_bass_kernel_spmd(nc, in_maps, list(range(NCORES))).results

    colsum = _CACHE["colsum"]            # [D] complex128
    out = np.empty((N, D), np.complex64)
    for cid in range(NCORES):
        o = np.asarray(res[cid]["out"], np.float32).reshape(D, NCH, 3, MC)
        B = o.transpose(2, 0, 1, 3).reshape(3, D, NS)      # [3, D, NS]
        corr_r = (B[0] - B[1]).T.astype(np.float64)        # [NS, D]
        corr_i = (B[2] - B[0] - B[1]).T.astype(np.float64)
        msg = (corr_r + 1j * corr_i) + colsum[None, :]
        rows = slice(cid * NS, (cid + 1) * NS)
        out[rows] = (msg / (np.abs(msg) + float(EPS))).astype(np.complex64)
    return out
